# revision 26
# baseline (speedup 1.0000x reference)
"""Trainium2 Bass kernel for a causal single-head attention block.

Reference computation (per batch b):
    q = x @ Wq + bq ; k = x @ Wk + bk ; v = x @ Wv + bv      (x: [S, D])
    logits = q @ k.T  (causal masked), probs = softmax(logits / sqrt(128))
    out = concat([x, probs @ v], axis=-1)                     -> [S, D+128]

Shapes are hardcoded: B=4, S=2048, D=1024, feature size 128, 8 NeuronCores.

Sharding (SPMD, one compiled graph for all 8 cores):
  core c -> batch b = c//2, interleave parity h = c%2.
  Each core computes the 8 query blocks (128 rows each) at global block
  positions {2j + h : j in 0..7} of its batch, and the K/V projection over
  the full 2048-row sequence of that batch.

  To keep the causal block structure identical across cores (SPMD requires
  one instruction stream), the host feeds h=1 cores a pair-swapped column
  order of x^T (global blocks [1,0,3,2,...]).  In local block coordinates
  every core then has: query blocks at even local positions 2j, with valid
  key blocks lk < 2j+2, a triangular causal mask on key slot lk=2j, and a
  slot lk=2j+1 that is fully masked for h=0 / fully valid for h=1.  The two
  128x128 mask tiles are per-core input data.

On-chip scheme:
  - host passes x^T as fp8 e4m3 [D, S] and the projection weights as fp8
    scaled by 32 (so their magnitudes sit in e4m3's normal range); the
    1/32 is folded into the PSUM-evacuation affine on DVE.
  - projections run as fp8 DoubleRow matmuls (2 contraction tiles per
    pass -> half the PE streaming time of bf16), fp32 PSUM, evacuated to
    bf16 kT/qT/vT with bias add on the vector engine.
  - logits computed transposed in bf16, q-chunk-major so the exp chain on
    the scalar engine (the longest serial non-DMA chain) starts as early
    as possible: logitsT[k, q] = kT_blk.T @ qT; after the masked exp,
    expT[k, q] is directly the stationary operand of the PV matmul.
  - v natural layout via 16 PE transposes of vT, augmented with a ones
    column: read_aug[q, 0:129] = expT.T @ [v | 1] accumulates both the
    attention read and the softmax denominators in one accumulation group
  - normalize with reciprocal * per-partition scale, write fp32
  - the x passthrough half of the output is a DRAM->DRAM SWDGE cast DMA
    from a bf16 copy of the core's own query rows (bf16 rounding is well
    inside the 2e-2 tolerance and halves the passthrough read traffic).

DMA plan: the two HWDGE rings (sync, scalar) carry the xT chunks in
parallel (two chunks each); the weight/constant loads ride the gpsimd
SWDGE ring so no trigger ever blocks the scalar engine's exp stream.  The
passthrough is held until the compute-critical xT chunks are done, then
saturates the remaining HBM bandwidth alongside the streamed read-part
writes.
"""

import math

import numpy as np
import ml_dtypes

import concourse.bass as bass
import concourse.tile as tile
from concourse import bacc, mybir
from concourse.bass_utils import run_bass_kernel_spmd

N_CORES = 8
B = 4
S = 2048  # sequence length per batch
D = 1024  # model dim
F = 128  # q/k/v feature size
NQT = 8  # local query subtiles of 128 rows
NKT = 16  # key tiles of 128 rows (full sequence)
QROWS = NQT * 128  # 1024 local query rows per core
SCALE = 1.0 / math.sqrt(F)
NEG = -1.0e9
WSCALE = 32.0  # fp8 weight prescale
# kT/qT/vT are kept scaled by 32 on chip (biases pre-scaled on the host);
# the 32*32 of the logits is folded into the exp scale, and the 32 of v
# cancels against a 32-valued ones column in the softmax denominator.
ESCALE = SCALE / (WSCALE * WSCALE)

FP32 = mybir.dt.float32
BF16 = mybir.dt.bfloat16
FP8 = mybir.dt.float8e4
U8 = mybir.dt.uint8
BF16_NP = ml_dtypes.bfloat16
FP8_NP = ml_dtypes.float8_e4m3
CBYTES = 3852  # packed per-partition constants: wk|wq|wv|masks|ident|bk|bq|bv

_compiled = {}

# xT DMA chunking (columns of the local sequence): chunk-major in DRAM so
# each chunk is one contiguous block.  Chunks alternate between the sync
# and scalar HWDGE rings so they land pairwise in parallel.
CHUNKS = tuple((256 * i, 256) for i in range(8))
DR = mybir.MatmulPerfMode.DoubleRow


def _build():
    nc = bacc.Bacc("TRN2", target_bir_lowering=False, debug=False, num_devices=N_CORES)

    xT_ext = nc.dram_tensor("xT", [D * S], FP8, kind="ExternalInput")
    xq_ext = nc.dram_tensor("xq", [QROWS, D], BF16, kind="ExternalInput")
    consts_ext = nc.dram_tensor("consts", [128, CBYTES], U8, kind="ExternalInput")
    out_ext = nc.dram_tensor("out", [QROWS, D + F], FP32, kind="ExternalOutput")

    with tile.TileContext(nc) as tc:
        with (
            tc.tile_pool(name="persist", bufs=1) as P,
            tc.tile_pool(name="ps_proj", bufs=3, space="PSUM") as ps_proj,
            tc.tile_pool(name="ps_log", bufs=2, space="PSUM") as ps_log,
            tc.tile_pool(name="ps_tp", bufs=1, space="PSUM") as ps_tp,
            tc.tile_pool(name="ps_read", bufs=2, space="PSUM") as ps_read,
        ):
            # ---- persistent SBUF tiles ----
            # chunk-major layout: [p, chunk, t, w] so each chunk DMA
            # writes one contiguous 2 KiB run per partition
            xT_sb = P.tile([128, 8, 8, 256], FP8)
            consts_sb = P.tile([128, CBYTES], U8)  # packed weights/constants
            wk_sb = consts_sb[:, 0:1024].bitcast(FP8).rearrange(
                "p (t f) -> p t f", t=8
            )
            wq_sb = consts_sb[:, 1024:2048].bitcast(FP8).rearrange(
                "p (t f) -> p t f", t=8
            )
            wv_sb = consts_sb[:, 2048:3072].bitcast(FP8).rearrange(
                "p (t f) -> p t f", t=8
            )
            mask_sb = consts_sb[:, 3072:3584].bitcast(BF16).rearrange(
                "p (s f) -> p s f", s=2
            )
            ident = consts_sb[:, 3584:3840].bitcast(BF16)
            bk_sb = consts_sb[:, 3840:3844].bitcast(FP32)
            bq_sb = consts_sb[:, 3844:3848].bitcast(FP32)
            bv_sb = consts_sb[:, 3848:3852].bitcast(FP32)
            kT_sb = P.tile([128, S], BF16)  # [feat, s]
            qT_sb = P.tile([128, QROWS], BF16)  # [feat, local q]
            vT_sb = P.tile([128, S], BF16)  # [feat, s]
            vaug_sb = P.tile([128, NKT, 132], BF16)  # [s%128, ki, vfeat|1]
            expT_sb = P.tile([128, NKT, QROWS], BF16)  # [s%128, ki, local q]
            read_sb = P.tile([128, NQT, 128], FP32)
            recip_sb = P.tile([128, NQT, 1], FP32)

            # ---- input DMAs.  Two HWDGE rings in parallel; per-ring FIFO
            # order puts what the first matmuls need first and the
            # passthrough-gating chunks last. ----
            UNIT = 128 * 8 * 256  # elements per 256-col chunk

            def chunk_dma(lo, n):
                # one DMA covering chunks [lo, lo+n): contiguous unit-major
                # DRAM, [p, c, t, w] SBUF view (per-partition contiguous)
                src = xT_ext[lo * UNIT:(lo + n) * UNIT].rearrange(
                    "(c p t w) -> p c t w", c=n, p=128, t=8
                )
                return nc.sync.dma_start(xT_sb[:, lo:lo + n, :, :], src)

            cd0 = chunk_dma(0, 1)
            # wk first so the very first matmul group is never gated on
            # the rest of the constants
            nc.scalar.dma_start(consts_sb[:, 0:1024], consts_ext[:, 0:1024])
            nc.scalar.dma_start(consts_sb[:, 1024:], consts_ext[:, 1024:])
            cd1 = chunk_dma(1, 1)
            cd23 = chunk_dma(2, 2)
            cd45 = chunk_dma(4, 2)
            cd67 = chunk_dma(6, 2)

            # ---- passthrough out[:, 0:D] = x rows: DRAM -> DRAM SWDGE
            # cast DMA (bf16 -> fp32), held until the xT loads finish.
            # Split four ways so Q7 descriptor generation pipelines with
            # the drain. ----
            for p in range(4):
                rows = slice(p * 256, (p + 1) * 256)
                pt_dma = nc.gpsimd.dma_start(
                    out=out_ext[rows, 0:D], in_=xq_ext[rows, :]
                )
                tile.add_dep_helper(
                    pt_dma.ins, cd23.ins, sync=True, reason="delay passthrough"
                )
                tile.add_dep_helper(
                    pt_dma.ins, cd45.ins, sync=True, reason="delay passthrough"
                )

            # ones column = 32 cancels the x32 scale of the v values
            nc.vector.memset(vaug_sb[:, :, 128:129], WSCALE)

            # ---- PE clock warmup: dummy matmuls on garbage SBUF keep the
            # tensor engine busy while the first inputs stream in, so the
            # real projections run at the warm 2.4 GHz clock. ----
            for _ in range(5):
                wm = ps_tp.tile([128, 256], FP32, tag="tp")
                nc.tensor.matmul(
                    wm[:], kT_sb[:, 0:128], kT_sb[:, 0:256], start=True, stop=True
                )

            # ---- projections (fp8 DoubleRow; outputs scaled x32) ----
            def proj_group(w_sb, b_sb, dst, ch, n):
                # one accumulation group over xT chunks [ch, ch+n)
                pp = ps_proj.tile([128, n * 256], FP32, tag="proj")
                mv = xT_sb[:, ch:ch + n, :, :].rearrange("p c t w -> p t c w")
                for u in range(4):
                    nc.tensor.matmul(
                        pp[:],
                        w_sb[:, 2 * u:2 * u + 2, :],
                        mv[:, 2 * u:2 * u + 2, :, :],
                        start=(u == 0),
                        stop=(u == 3),
                        perf_mode=DR,
                    )
                nc.vector.tensor_scalar(
                    dst[:, ch * 256:(ch + n) * 256], pp[:], b_sb, None,
                    mybir.AluOpType.add,
                )

            def q_quarter(qq):
                # q rows = even local 128-col blocks; 256-col chunk j holds
                # local q block j in its even half
                qv = xT_sb[:, 2 * qq:2 * qq + 2, :, :].rearrange(
                    "p c t (two f) -> p t c two f", two=2
                )
                pp = ps_proj.tile([128, 256], FP32, tag="proj")
                for u in range(4):
                    nc.tensor.matmul(
                        pp[:],
                        wq_sb[:, 2 * u:2 * u + 2, :],
                        qv[:, 2 * u:2 * u + 2, :, 0, :],
                        start=(u == 0),
                        stop=(u == 3),
                        perf_mode=DR,
                    )
                nc.vector.tensor_scalar(
                    qT_sb[:, qq * 256:(qq + 1) * 256], pp[:], bq_sb, None,
                    mybir.AluOpType.add,
                )

            def v_transpose4(k0):
                # transpose 4 key tiles into one PSUM tile, one batched copy
                pt = ps_tp.tile([128, 4, 128], BF16, tag="tp")
                for i in range(4):
                    ki = k0 + i
                    nc.tensor.transpose(
                        pt[:, i, :], vT_sb[:, ki * 128:(ki + 1) * 128], ident[:]
                    )
                nc.vector.tensor_copy(vaug_sb[:, k0:k0 + 4, 0:128], pt[:])

            def logits_exp(ki, lo, hi):
                # logits^T for key tile ki over local query columns [lo,hi)
                qs = 128 * (ki // 2)
                off = max(lo, qs)
                if off >= hi:
                    return
                w = hi - off
                kb = slice(ki * 128, (ki + 1) * 128)
                pl = ps_log.tile([128, w], FP32, tag="log")
                nc.tensor.matmul(
                    pl[:], kT_sb[:, kb], qT_sb[:, off:off + w],
                    start=True, stop=True,
                )
                if off == qs:  # this range contains the diagonal block
                    nc.vector.tensor_add(
                        pl[:, 0:128], pl[:, 0:128], mask_sb[:, ki % 2, :]
                    )
                nc.scalar.activation(
                    expT_sb[:, ki, off:off + w], pl[:],
                    mybir.ActivationFunctionType.Exp, scale=ESCALE,
                )

            def pv(j):
                pr = ps_read.tile([128, 129], FP32, tag="read")
                last = 2 * j + 1
                for ki in range(last + 1):
                    nc.tensor.matmul(
                        pr[:],
                        expT_sb[:, ki, j * 128:(j + 1) * 128],
                        vaug_sb[:, ki, 0:129],
                        start=(ki == 0),
                        stop=(ki == last),
                    )
                nc.vector.reciprocal(recip_sb[:, j, :], pr[:, 128:129])
                nc.vector.tensor_scalar_mul(
                    read_sb[:, j, :], pr[:, 0:128], recip_sb[:, j, :]
                )
                out_read = out_ext[:].rearrange("(g p) c -> p g c", p=128)
                nc.sync.dma_start(
                    out=out_read[:, j, D:D + F], in_=read_sb[:, j, :]
                )

            # ---- pipeline order (per-engine stream order = program
            # order).  The exp chain on the scalar engine is the longest
            # serial compute chain; logits tiles are interleaved into the
            # projection stream so it starts as early as possible and
            # never starves. ----
            proj_group(wk_sb, bk_sb, kT_sb, 0, 1)                # chunk 0
            proj_group(wk_sb, bk_sb, kT_sb, 1, 1)                # chunk 1
            q_quarter(0)                                         # chunks 0,1
            q_quarter(1)                                         # chunks 2,3
            logits_exp(0, 0, 512)
            logits_exp(1, 0, 512)
            proj_group(wk_sb, bk_sb, kT_sb, 2, 2)                # chunks 2,3
            logits_exp(2, 0, 512)
            logits_exp(3, 0, 512)
            logits_exp(4, 0, 512)
            logits_exp(5, 0, 512)
            logits_exp(6, 0, 512)
            logits_exp(7, 0, 512)
            q_quarter(2)                                         # chunks 4,5
            q_quarter(3)                                         # chunks 6,7
            proj_group(wk_sb, bk_sb, kT_sb, 4, 2)                # chunks 4,5
            proj_group(wk_sb, bk_sb, kT_sb, 6, 2)                # chunks 6,7
            logits_exp(0, 512, 1024)
            logits_exp(1, 512, 1024)
            proj_group(wv_sb, bv_sb, vT_sb, 0, 2)
            logits_exp(2, 512, 1024)
            logits_exp(3, 512, 1024)
            proj_group(wv_sb, bv_sb, vT_sb, 2, 2)
            logits_exp(4, 512, 1024)
            logits_exp(5, 512, 1024)
            proj_group(wv_sb, bv_sb, vT_sb, 4, 2)
            logits_exp(6, 512, 1024)
            logits_exp(7, 512, 1024)
            proj_group(wv_sb, bv_sb, vT_sb, 6, 2)
            logits_exp(8, 512, 1024)
            logits_exp(9, 512, 1024)
            v_transpose4(0)
            v_transpose4(4)
            logits_exp(10, 512, 1024)
            logits_exp(11, 512, 1024)
            v_transpose4(8)
            v_transpose4(12)
            logits_exp(12, 512, 1024)
            logits_exp(13, 512, 1024)
            pv(0)
            pv(1)
            logits_exp(14, 512, 1024)
            logits_exp(15, 512, 1024)
            for j in range(2, NQT):
                pv(j)

    nc.compile()
    return nc


def _get_compiled():
    if "nc" not in _compiled:
        _compiled["nc"] = _build()
    return _compiled["nc"]


def _to_fp8(a):
    return np.clip(np.asarray(a, np.float32), -240.0, 240.0).astype(FP8_NP)


def _make_in_maps(inputs, Wq, bq, Wk, bk, Wv, bv):
    x = np.asarray(inputs, dtype=np.float32)
    assert x.shape == (B, S, D)

    def prep_w(w):
        w = _to_fp8(np.asarray(w, np.float32) * WSCALE)
        return np.ascontiguousarray(w.reshape(8, 128, 128).transpose(1, 0, 2))

    wq_np, wk_np, wv_np = prep_w(Wq), prep_w(Wk), prep_w(Wv)
    bq_np = np.asarray(bq, np.float32).reshape(128, 1)
    bk_np = np.asarray(bk, np.float32).reshape(128, 1)
    bv_np = np.asarray(bv, np.float32).reshape(128, 1)
    ident_np = np.eye(128, dtype=BF16_NP)

    # masks[k, slot, q]: slot 0 = diagonal block (triangular), slot 1 = the
    # extra block (fully masked for h=0, fully valid for h=1)
    kk = np.arange(128)[:, None]
    qq = np.arange(128)[None, :]
    tri = np.where(qq >= kk, 0.0, NEG).astype(np.float32)
    consts_h = []
    for h in range(2):
        other = np.full((128, 128), NEG if h == 0 else 0.0, np.float32)
        m = np.stack([tri, other], axis=1).astype(BF16_NP)  # [k, slot, q]
        cb = np.zeros((128, CBYTES), np.uint8)
        cb[:, 0:1024] = wk_np.reshape(128, 1024).view(np.uint8)
        cb[:, 1024:2048] = wq_np.reshape(128, 1024).view(np.uint8)
        cb[:, 2048:3072] = wv_np.reshape(128, 1024).view(np.uint8)
        cb[:, 3072:3584] = np.ascontiguousarray(m.reshape(128, 256)).view(np.uint8)
        cb[:, 3584:3840] = ident_np.view(np.uint8)
        cb[:, 3840:3844] = (bk_np * WSCALE).view(np.uint8)
        cb[:, 3844:3848] = (bq_np * WSCALE).view(np.uint8)
        cb[:, 3848:3852] = (bv_np * WSCALE).view(np.uint8)
        consts_h.append(cb)

    in_maps = []
    for c in range(N_CORES):
        b, h = divmod(c, 2)
        xb = x[b]  # [S, D]
        # local block order: pair-swap for h=1
        order = np.arange(NKT) if h == 0 else (np.arange(NKT) ^ 1)
        xb_local = xb.reshape(NKT, 128, D)[order].reshape(S, D)
        xT_full = _to_fp8(xb_local.T)  # [D, S] = [(t p), s]
        xT_tps = xT_full.reshape(8, 128, S).transpose(1, 0, 2)  # [p, t, s]
        xT = np.concatenate(
            [xT_tps[:, :, off:off + w].reshape(-1) for off, w in CHUNKS]
        )  # chunk-major flat, each chunk contiguous [p, t, w]
        # own query rows = even local blocks = global blocks 2j+h
        qrows = xb_local.reshape(NKT, 128, D)[0::2].reshape(QROWS, D)
        xq = np.ascontiguousarray(qrows).astype(BF16_NP)
        in_maps.append({"xT": xT, "xq": xq, "consts": consts_h[h]})
    return in_maps


def _gather(results):
    out = np.empty((B, S, D + F), dtype=np.float32)
    for c in range(N_CORES):
        b, h = divmod(c, 2)
        oc = results[c]["out"].reshape(NQT, 128, D + F)
        for j in range(NQT):
            g = 2 * j + h
            out[b, g * 128:(g + 1) * 128, :] = oc[j]
    return out


def run(inputs, Wq, bq, Wk, bk, Wv, bv, trace=False, tmpdir=None):
    """Build (cached), run on 8 cores, gather. Returns (output, results)."""
    nc = _get_compiled()
    in_maps = _make_in_maps(inputs, Wq, bq, Wk, bk, Wv, bv)
    if trace:
        try:
            res = run_bass_kernel_spmd(
                nc, in_maps, list(range(N_CORES)), trace=True, tmpdir=tmpdir
            )
            return _gather(res.results), res
        except Exception as e:  # profiling hook unavailable etc.
            print(f"trace run failed ({e!r}); falling back to untraced run")
    res = run_bass_kernel_spmd(nc, in_maps, list(range(N_CORES)))
    return _gather(res.results), res


def kernel(inputs, Wq, bq, Wk, bk, Wv, bv):
    out, _ = run(inputs, Wq, bq, Wk, bk, Wv, bv, trace=False)
    return out


# revision 28
# speedup vs baseline: 1.0015x; 1.0015x over previous
"""Trainium2 Bass kernel for a causal single-head attention block.

Reference computation (per batch b):
    q = x @ Wq + bq ; k = x @ Wk + bk ; v = x @ Wv + bv      (x: [S, D])
    logits = q @ k.T  (causal masked), probs = softmax(logits / sqrt(128))
    out = concat([x, probs @ v], axis=-1)                     -> [S, D+128]

Shapes are hardcoded: B=4, S=2048, D=1024, feature size 128, 8 NeuronCores.

Sharding (SPMD, one compiled graph for all 8 cores):
  core c -> batch b = c//2, interleave parity h = c%2.
  Each core computes the 8 query blocks (128 rows each) at global block
  positions {2j + h : j in 0..7} of its batch, and the K/V projection over
  the full 2048-row sequence of that batch.

  To keep the causal block structure identical across cores (SPMD requires
  one instruction stream), the host feeds h=1 cores a pair-swapped column
  order of x^T (global blocks [1,0,3,2,...]).  In local block coordinates
  every core then has: query blocks at even local positions 2j, with valid
  key blocks lk < 2j+2, a triangular causal mask on key slot lk=2j, and a
  slot lk=2j+1 that is fully masked for h=0 / fully valid for h=1.  The two
  128x128 mask tiles are per-core input data.

On-chip scheme:
  - host passes x^T as fp8 e4m3 [D, S] and the projection weights as fp8
    scaled by 32 (so their magnitudes sit in e4m3's normal range); the
    1/32 is folded into the PSUM-evacuation affine on DVE.
  - projections run as fp8 DoubleRow matmuls (2 contraction tiles per
    pass -> half the PE streaming time of bf16), fp32 PSUM, evacuated to
    bf16 kT/qT/vT with bias add on the vector engine.
  - logits computed transposed in bf16, q-chunk-major so the exp chain on
    the scalar engine (the longest serial non-DMA chain) starts as early
    as possible: logitsT[k, q] = kT_blk.T @ qT; after the masked exp,
    expT[k, q] is directly the stationary operand of the PV matmul.
  - v natural layout via 16 PE transposes of vT, augmented with a ones
    column: read_aug[q, 0:129] = expT.T @ [v | 1] accumulates both the
    attention read and the softmax denominators in one accumulation group
  - normalize with reciprocal * per-partition scale, write fp32
  - the x passthrough half of the output is a DRAM->DRAM SWDGE cast DMA
    from a bf16 copy of the core's own query rows (bf16 rounding is well
    inside the 2e-2 tolerance and halves the passthrough read traffic).

DMA plan: the two HWDGE rings (sync, scalar) carry the xT chunks in
parallel (two chunks each); the weight/constant loads ride the gpsimd
SWDGE ring so no trigger ever blocks the scalar engine's exp stream.  The
passthrough is held until the compute-critical xT chunks are done, then
saturates the remaining HBM bandwidth alongside the streamed read-part
writes.
"""

import math

import numpy as np
import ml_dtypes

import concourse.bass as bass
import concourse.tile as tile
from concourse import bacc, mybir
from concourse.bass_utils import run_bass_kernel_spmd

N_CORES = 8
B = 4
S = 2048  # sequence length per batch
D = 1024  # model dim
F = 128  # q/k/v feature size
NQT = 8  # local query subtiles of 128 rows
NKT = 16  # key tiles of 128 rows (full sequence)
QROWS = NQT * 128  # 1024 local query rows per core
SCALE = 1.0 / math.sqrt(F)
NEG = -1.0e9
WSCALE = 32.0  # fp8 weight prescale
# kT/qT/vT are kept scaled by 32 on chip (biases pre-scaled on the host);
# the 32*32 of the logits is folded into the exp scale, and the 32 of v
# cancels against a 32-valued ones column in the softmax denominator.
ESCALE = SCALE / (WSCALE * WSCALE)

FP32 = mybir.dt.float32
BF16 = mybir.dt.bfloat16
FP8 = mybir.dt.float8e4
U8 = mybir.dt.uint8
BF16_NP = ml_dtypes.bfloat16
FP8_NP = ml_dtypes.float8_e4m3
CBYTES = 3852  # packed per-partition constants: wk|wq|wv|masks|ident|bk|bq|bv

_compiled = {}

# xT DMA chunking (columns of the local sequence): chunk-major in DRAM so
# each chunk is one contiguous block.  Chunks alternate between the sync
# and scalar HWDGE rings so they land pairwise in parallel.
CHUNKS = ((0, 512), (512, 512), (1024, 512), (1536, 512))
DR = mybir.MatmulPerfMode.DoubleRow


def _build():
    nc = bacc.Bacc("TRN2", target_bir_lowering=False, debug=False, num_devices=N_CORES)

    xT_ext = nc.dram_tensor("xT", [D * S], FP8, kind="ExternalInput")
    xq_ext = nc.dram_tensor("xq", [QROWS, D], BF16, kind="ExternalInput")
    consts_ext = nc.dram_tensor("consts", [128, CBYTES], U8, kind="ExternalInput")
    out_ext = nc.dram_tensor("out", [QROWS, D + F], FP32, kind="ExternalOutput")

    with tile.TileContext(nc) as tc:
        with (
            tc.tile_pool(name="persist", bufs=1) as P,
            tc.tile_pool(name="ps_proj", bufs=2, space="PSUM") as ps_proj,
            tc.tile_pool(name="ps_log", bufs=3, space="PSUM") as ps_log,
            tc.tile_pool(name="ps_tp", bufs=1, space="PSUM") as ps_tp,
            tc.tile_pool(name="ps_read", bufs=2, space="PSUM") as ps_read,
        ):
            # ---- persistent SBUF tiles ----
            # chunk-major layout: [p, chunk, t, w] so each chunk DMA
            # writes one contiguous 4 KiB run per partition
            xT_sb = P.tile([128, 4, 8, 512], FP8)
            consts_sb = P.tile([128, CBYTES], U8)  # packed weights/constants
            wk_sb = consts_sb[:, 0:1024].bitcast(FP8).rearrange(
                "p (t f) -> p t f", t=8
            )
            wq_sb = consts_sb[:, 1024:2048].bitcast(FP8).rearrange(
                "p (t f) -> p t f", t=8
            )
            wv_sb = consts_sb[:, 2048:3072].bitcast(FP8).rearrange(
                "p (t f) -> p t f", t=8
            )
            mask_sb = consts_sb[:, 3072:3584].bitcast(BF16).rearrange(
                "p (s f) -> p s f", s=2
            )
            ident = consts_sb[:, 3584:3840].bitcast(BF16)
            bk_sb = consts_sb[:, 3840:3844].bitcast(FP32)
            bq_sb = consts_sb[:, 3844:3848].bitcast(FP32)
            bv_sb = consts_sb[:, 3848:3852].bitcast(FP32)
            kT_sb = P.tile([128, S], BF16)  # [feat, s]
            qT_sb = P.tile([128, QROWS], BF16)  # [feat, local q]
            vT_sb = P.tile([128, S], BF16)  # [feat, s]
            vaug_sb = P.tile([128, NKT, 132], BF16)  # [s%128, ki, vfeat|1]
            expT_sb = P.tile([128, NKT, QROWS], BF16)  # [s%128, ki, local q]
            read_sb = P.tile([128, NQT, 128], FP32)
            recip_sb = P.tile([128, NQT, 1], FP32)

            # ---- input DMAs.  Two HWDGE rings in parallel; per-ring FIFO
            # order puts what the first matmuls need first and the
            # passthrough-gating chunks last. ----
            UNIT = 128 * 8 * 512  # elements per 512-col chunk

            def chunk_dma(i):
                src = xT_ext[i * UNIT:(i + 1) * UNIT].rearrange(
                    "(p t w) -> p t w", p=128, t=8
                )
                return nc.sync.dma_start(xT_sb[:, i, :, :], src)

            c0 = chunk_dma(0)
            # wk first so the very first matmul group is never gated on
            # the rest of the constants
            nc.scalar.dma_start(consts_sb[:, 0:1024], consts_ext[:, 0:1024])
            nc.scalar.dma_start(consts_sb[:, 1024:], consts_ext[:, 1024:])
            c1 = chunk_dma(1)
            c2 = chunk_dma(2)
            c3 = chunk_dma(3)

            # ---- passthrough out[:, 0:D] = x rows: DRAM -> DRAM SWDGE
            # cast DMA (bf16 -> fp32), held until the xT loads finish.
            # Split four ways so Q7 descriptor generation pipelines with
            # the drain. ----
            for p in range(4):
                rows = slice(p * 256, (p + 1) * 256)
                pt_dma = nc.gpsimd.dma_start(
                    out=out_ext[rows, 0:D], in_=xq_ext[rows, :]
                )
                tile.add_dep_helper(
                    pt_dma.ins, c1.ins, sync=True, reason="delay passthrough"
                )
                tile.add_dep_helper(
                    pt_dma.ins, c2.ins, sync=True, reason="delay passthrough"
                )

            # ones column = 32 cancels the x32 scale of the v values
            nc.vector.memset(vaug_sb[:, :, 128:129], WSCALE)

            # ---- PE clock warmup: dummy matmuls on garbage SBUF keep the
            # tensor engine busy while the first inputs stream in, so the
            # real projections run at the warm 2.4 GHz clock. ----
            for _ in range(10):
                wm = ps_tp.tile([128, 256], FP32, tag="tp")
                nc.tensor.matmul(
                    wm[:], kT_sb[:, 0:128], kT_sb[:, 0:256], start=True, stop=True
                )

            # ---- projections (fp8 DoubleRow; outputs scaled x32) ----
            def proj_chunk(w_sb, b_sb, dst, c):
                pp = ps_proj.tile([128, 512], FP32, tag="proj")
                for u in range(4):
                    nc.tensor.matmul(
                        pp[:],
                        w_sb[:, 2 * u:2 * u + 2, :],
                        xT_sb[:, c, 2 * u:2 * u + 2, :],
                        start=(u == 0),
                        stop=(u == 3),
                        perf_mode=DR,
                    )
                nc.vector.tensor_scalar(
                    dst[:, c * 512:(c + 1) * 512], pp[:], b_sb, None,
                    mybir.AluOpType.add,
                )

            def q_quarter(qq):
                # q rows = even local 128-col blocks; 512-col chunk qq holds
                # local q blocks 2qq, 2qq+1 at its even sub-positions
                qv = xT_sb[:, qq, :, :].rearrange(
                    "p t (g two f) -> p t g two f", two=2, f=128
                )
                pp = ps_proj.tile([128, 256], FP32, tag="proj")
                for u in range(4):
                    nc.tensor.matmul(
                        pp[:],
                        wq_sb[:, 2 * u:2 * u + 2, :],
                        qv[:, 2 * u:2 * u + 2, :, 0, :],
                        start=(u == 0),
                        stop=(u == 3),
                        perf_mode=DR,
                    )
                nc.vector.tensor_scalar(
                    qT_sb[:, qq * 256:(qq + 1) * 256], pp[:], bq_sb, None,
                    mybir.AluOpType.add,
                )

            def v_transpose4(k0):
                # transpose 4 key tiles into one PSUM tile, one batched copy
                pt = ps_tp.tile([128, 4, 128], BF16, tag="tp")
                for i in range(4):
                    ki = k0 + i
                    nc.tensor.transpose(
                        pt[:, i, :], vT_sb[:, ki * 128:(ki + 1) * 128], ident[:]
                    )
                nc.vector.tensor_copy(vaug_sb[:, k0:k0 + 4, 0:128], pt[:])

            def logits_exp(ki, lo, hi):
                # logits^T for key tile ki over local query columns [lo,hi)
                qs = 128 * (ki // 2)
                off = max(lo, qs)
                if off >= hi:
                    return
                w = hi - off
                kb = slice(ki * 128, (ki + 1) * 128)
                pl = ps_log.tile([128, w], FP32, tag="log")
                nc.tensor.matmul(
                    pl[:], kT_sb[:, kb], qT_sb[:, off:off + w],
                    start=True, stop=True,
                )
                if off == qs:  # this range contains the diagonal block
                    nc.vector.tensor_add(
                        pl[:, 0:128], pl[:, 0:128], mask_sb[:, ki % 2, :]
                    )
                nc.scalar.activation(
                    expT_sb[:, ki, off:off + w], pl[:],
                    mybir.ActivationFunctionType.Exp, scale=ESCALE,
                )

            def pv(j):
                pr = ps_read.tile([128, 129], FP32, tag="read")
                last = 2 * j + 1
                for ki in range(last + 1):
                    nc.tensor.matmul(
                        pr[:],
                        expT_sb[:, ki, j * 128:(j + 1) * 128],
                        vaug_sb[:, ki, 0:129],
                        start=(ki == 0),
                        stop=(ki == last),
                    )
                nc.vector.reciprocal(recip_sb[:, j, :], pr[:, 128:129])
                nc.vector.tensor_scalar_mul(
                    read_sb[:, j, :], pr[:, 0:128], recip_sb[:, j, :]
                )
                out_read = out_ext[:].rearrange("(g p) c -> p g c", p=128)
                nc.sync.dma_start(
                    out=out_read[:, j, D:D + F], in_=read_sb[:, j, :]
                )

            # ---- pipeline order (per-engine stream order = program
            # order).  The exp chain on the scalar engine is the longest
            # serial compute chain; logits tiles are interleaved into the
            # projection stream so it starts as early as possible and
            # never starves. ----
            proj_chunk(wk_sb, bk_sb, kT_sb, 0)                   # needs c0
            q_quarter(0)                                         # needs c0
            q_quarter(1)                                         # needs c1
            logits_exp(0, 0, 512)
            logits_exp(1, 0, 512)
            logits_exp(2, 0, 512)
            logits_exp(3, 0, 512)
            proj_chunk(wk_sb, bk_sb, kT_sb, 1)                   # needs c1
            logits_exp(4, 0, 512)
            logits_exp(5, 0, 512)
            logits_exp(6, 0, 512)
            logits_exp(7, 0, 512)
            q_quarter(2)                                         # needs c2
            q_quarter(3)                                         # needs c3
            logits_exp(0, 512, 1024)
            logits_exp(1, 512, 1024)
            logits_exp(2, 512, 1024)
            logits_exp(3, 512, 1024)
            proj_chunk(wk_sb, bk_sb, kT_sb, 2)                   # needs c2
            logits_exp(4, 512, 1024)
            logits_exp(5, 512, 1024)
            proj_chunk(wk_sb, bk_sb, kT_sb, 3)                   # needs c3
            logits_exp(6, 512, 1024)
            logits_exp(7, 512, 1024)
            logits_exp(8, 512, 1024)
            logits_exp(9, 512, 1024)
            proj_chunk(wv_sb, bv_sb, vT_sb, 0)
            logits_exp(10, 512, 1024)
            logits_exp(11, 512, 1024)
            proj_chunk(wv_sb, bv_sb, vT_sb, 1)
            logits_exp(12, 512, 1024)
            logits_exp(13, 512, 1024)
            proj_chunk(wv_sb, bv_sb, vT_sb, 2)
            logits_exp(14, 512, 1024)
            logits_exp(15, 512, 1024)
            proj_chunk(wv_sb, bv_sb, vT_sb, 3)
            v_transpose4(0)
            v_transpose4(4)
            v_transpose4(8)
            v_transpose4(12)
            for j in range(NQT):
                pv(j)

    nc.compile()
    return nc


def _get_compiled():
    if "nc" not in _compiled:
        _compiled["nc"] = _build()
    return _compiled["nc"]


def _to_fp8(a):
    return np.clip(np.asarray(a, np.float32), -240.0, 240.0).astype(FP8_NP)


def _make_in_maps(inputs, Wq, bq, Wk, bk, Wv, bv):
    x = np.asarray(inputs, dtype=np.float32)
    assert x.shape == (B, S, D)

    def prep_w(w):
        w = _to_fp8(np.asarray(w, np.float32) * WSCALE)
        return np.ascontiguousarray(w.reshape(8, 128, 128).transpose(1, 0, 2))

    wq_np, wk_np, wv_np = prep_w(Wq), prep_w(Wk), prep_w(Wv)
    bq_np = np.asarray(bq, np.float32).reshape(128, 1)
    bk_np = np.asarray(bk, np.float32).reshape(128, 1)
    bv_np = np.asarray(bv, np.float32).reshape(128, 1)
    ident_np = np.eye(128, dtype=BF16_NP)

    # masks[k, slot, q]: slot 0 = diagonal block (triangular), slot 1 = the
    # extra block (fully masked for h=0, fully valid for h=1)
    kk = np.arange(128)[:, None]
    qq = np.arange(128)[None, :]
    tri = np.where(qq >= kk, 0.0, NEG).astype(np.float32)
    consts_h = []
    for h in range(2):
        other = np.full((128, 128), NEG if h == 0 else 0.0, np.float32)
        m = np.stack([tri, other], axis=1).astype(BF16_NP)  # [k, slot, q]
        cb = np.zeros((128, CBYTES), np.uint8)
        cb[:, 0:1024] = wk_np.reshape(128, 1024).view(np.uint8)
        cb[:, 1024:2048] = wq_np.reshape(128, 1024).view(np.uint8)
        cb[:, 2048:3072] = wv_np.reshape(128, 1024).view(np.uint8)
        cb[:, 3072:3584] = np.ascontiguousarray(m.reshape(128, 256)).view(np.uint8)
        cb[:, 3584:3840] = ident_np.view(np.uint8)
        cb[:, 3840:3844] = (bk_np * WSCALE).view(np.uint8)
        cb[:, 3844:3848] = (bq_np * WSCALE).view(np.uint8)
        cb[:, 3848:3852] = (bv_np * WSCALE).view(np.uint8)
        consts_h.append(cb)

    in_maps = []
    for c in range(N_CORES):
        b, h = divmod(c, 2)
        xb = x[b]  # [S, D]
        # local block order: pair-swap for h=1
        order = np.arange(NKT) if h == 0 else (np.arange(NKT) ^ 1)
        xb_local = xb.reshape(NKT, 128, D)[order].reshape(S, D)
        xT_full = _to_fp8(xb_local.T)  # [D, S] = [(t p), s]
        xT_tps = xT_full.reshape(8, 128, S).transpose(1, 0, 2)  # [p, t, s]
        xT = np.concatenate(
            [xT_tps[:, :, off:off + w].reshape(-1) for off, w in CHUNKS]
        )  # chunk-major flat, each chunk contiguous [p, t, w]
        # own query rows = even local blocks = global blocks 2j+h
        qrows = xb_local.reshape(NKT, 128, D)[0::2].reshape(QROWS, D)
        xq = np.ascontiguousarray(qrows).astype(BF16_NP)
        in_maps.append({"xT": xT, "xq": xq, "consts": consts_h[h]})
    return in_maps


def _gather(results):
    out = np.empty((B, S, D + F), dtype=np.float32)
    for c in range(N_CORES):
        b, h = divmod(c, 2)
        oc = results[c]["out"].reshape(NQT, 128, D + F)
        for j in range(NQT):
            g = 2 * j + h
            out[b, g * 128:(g + 1) * 128, :] = oc[j]
    return out


def run(inputs, Wq, bq, Wk, bk, Wv, bv, trace=False, tmpdir=None):
    """Build (cached), run on 8 cores, gather. Returns (output, results)."""
    nc = _get_compiled()
    in_maps = _make_in_maps(inputs, Wq, bq, Wk, bk, Wv, bv)
    if trace:
        try:
            res = run_bass_kernel_spmd(
                nc, in_maps, list(range(N_CORES)), trace=True, tmpdir=tmpdir
            )
            return _gather(res.results), res
        except Exception as e:  # profiling hook unavailable etc.
            print(f"trace run failed ({e!r}); falling back to untraced run")
    res = run_bass_kernel_spmd(nc, in_maps, list(range(N_CORES)))
    return _gather(res.results), res


def kernel(inputs, Wq, bq, Wk, bk, Wv, bv):
    out, _ = run(inputs, Wq, bq, Wk, bk, Wv, bv, trace=False)
    return out
